# revision 4
# baseline (speedup 1.0000x reference)
"""Trainium2 Bass kernel for nn_DistanceDecoder (moe_routing).

reference:
    comp_b  = components[object_labels]            # [B, 32, 6144]
    mean_b  = means[object_labels]                 # [B, 6144]
    out     = einsum('bp,bpo->bo', lattent, comp_b) + mean_b

Strategy (8 NeuronCores):
  * Shard OUT_DIM (6144) 8-ways -> each core owns a 768-wide column slice
    and the full batch.  Per-core HBM traffic is then ~2.8 MB (its own
    fp16 slice of the PCA table + 1.5 MB fp16 output) instead of the
    18 MB the batch-parallel split would need.
  * On the host, stable-sort the batch by label (MoE dispatch) and append
    the per-object mean as a 33rd row of each object's [32, 768] component
    block with a matching constant-1.0 row in the latent matrix, so gather
    + vecmat + mean-add is a single block-banded matmul
        out_T[768, 1024] = C2aug^T @ Epack
    over 7 K-tiles of 3 objects (K = 3*33 = 99 rows).  After the sort,
    each K-tile's samples form one contiguous column range, baked into the
    instruction stream as matmul free-dim offsets.
  * Matmul operands are fp16 (~10-bit-mantissa rounding, full-rate PE,
    half the DMA bytes).  The output is also stored fp16 (cast during the
    PSUM->SBUF drain, upcast on the host): adds ~2e-4 rel err and halves
    the store traffic, which runs at the chip HBM bound with all 8 cores
    storing at once.
  * DMA descriptor sizes are kept >= ~4KB where possible (HBM needs
    ~4KB/descriptor to reach line rate): comp is loaded in 3 two-chunk
    DMAs (3.5KB/partition-row), and the output DRAM tensor is laid out
    interleaved as [128, NCHUNK*1024] (row r holds row r of every chunk)
    so a two-chunk store is one DMA with 4KB rows.  Profiled v1 with
    per-chunk loads/stores (1.8-2KB rows) sustained only ~150-300 GB/s
    per core vs ~410 GB/s for 4KB rows.
  * DMA work is split across the two HWDGE rings (FIFO per ring):
    sync(SP) carries comp{0,1} + stores for chunks 2-5; scalar(ACT)
    carries epack, comp{2,3}, comp{4,5} + the chunk{0,1} store placed
    where it cannot block a later PSUM drain on the ACT queue.
  * PSUM drains are one [128,1024] f32->fp16 copy per chunk (2 PSUM
    banks), alternating DVE / ACT so the two PSUM-capable engines halve
    the drain serially; the last chunk drains per 512-segment so its
    store can start after the first half.
  * DD_DTYPE=f32r swaps in fp32r (full fp32 operand bits, f32 output);
    fp32r matmuls then require even range starts/widths, fixed by zero
    pad columns, with samples pushed past column 1024 computed on the
    host.
  * Host applies the inverse permutation / column concat at the end.
"""

import os

import numpy as np

BATCH = 1024
PCA = 32
ROWS = PCA + 1             # 32 components + 1 mean row per object
OUT_DIM = 6144
NOBJ = 20
NCORES = 8
SLICE = OUT_DIM // NCORES  # 768
NCHUNK = SLICE // 128      # 6 chunks of 128 output rows (out_T partitions)
OBJ_PER_KT = 3             # objects per K-tile -> K = 3*33 = 99 <= 128
KTILES = (NOBJ + OBJ_PER_KT - 1) // OBJ_PER_KT  # 7
KP = OBJ_PER_KT * ROWS     # 99 partitions per K-tile
SEGS = [(0, 512), (512, 1024)]  # PSUM bank segments

DTYPE = os.environ.get("DD_DTYPE", "fp16")  # "fp16" | "f32r"

_NC_CACHE: dict = {}


def _kheight(t: int) -> int:
    return (min(OBJ_PER_KT * (t + 1), NOBJ) - OBJ_PER_KT * t) * ROWS


def _np_dtype():
    return np.float16 if DTYPE == "fp16" else np.float32


def _build_nc(ranges: tuple):
    """Build + compile the single-core Bass program (SPMD across 8 cores).

    ranges: KTILES+1 ints; ranges[t]..ranges[t+1] is the sorted-batch column
    range whose labels fall in objects [3t, 3t+3) — baked into the
    instruction stream as matmul free-dim offsets.
    """
    import concourse.mybir as mybir
    from concourse import bacc
    from concourse.tile import TileContext

    dt_in = mybir.dt.float16 if DTYPE == "fp16" else mybir.dt.float32r
    dt_out = mybir.dt.float16 if DTYPE == "fp16" else mybir.dt.float32
    f32 = mybir.dt.float32

    nc = bacc.Bacc("TRN2", target_bir_lowering=False, debug=False)

    # cols of comp: j*(KTILES*128) + t*128 + m   (j = out_T row chunk)
    CCOLS = KTILES * 128
    comp_d = nc.dram_tensor(
        "comp", [KP, NCHUNK * CCOLS], dt_in, kind="ExternalInput"
    )
    epack_d = nc.dram_tensor("epack", [KP, BATCH], dt_in, kind="ExternalInput")
    # interleaved: out_d[r, j*BATCH + c] = out_T[j*128 + r, c]
    out_d = nc.dram_tensor(
        "out", [128, NCHUNK * BATCH], dt_out, kind="ExternalOutput"
    )

    with TileContext(nc) as tc:
        with (
            tc.tile_pool(name="const", bufs=1) as cpool,
            tc.tile_pool(name="outp", bufs=1) as opool,
            tc.tile_pool(name="ps", bufs=4, space="PSUM") as pspool,
        ):
            # inputs: comp{0,1} on the SP ring; epack then comp{2,3},
            # comp{4,5} on the ACT ring (epack descriptors drain first)
            cpair = []
            for p in range(NCHUNK // 2):
                cp = cpool.tile([KP, 2 * CCOLS], dt_in, name=f"cpair{p}")
                cpair.append(cp)
            nc.sync.dma_start(out=cpair[0], in_=comp_d[:, 0 : 2 * CCOLS])
            epack = cpool.tile([KP, BATCH], dt_in)
            nc.scalar.dma_start(out=epack, in_=epack_d[:, :])
            nc.scalar.dma_start(
                out=cpair[1], in_=comp_d[:, 2 * CCOLS : 4 * CCOLS]
            )
            nc.scalar.dma_start(
                out=cpair[2], in_=comp_d[:, 4 * CCOLS : 6 * CCOLS]
            )

            # output staging: two-chunk pair tiles (stored as one 4KB-row
            # DMA) for chunks 0-3; chunks 4,5 single, 5 split per segment
            p01 = opool.tile([128, 2 * BATCH], dt_out, name="p01")
            p23 = opool.tile([128, 2 * BATCH], dt_out, name="p23")
            o4 = opool.tile([128, BATCH], dt_out, name="o4")
            o5 = opool.tile([128, BATCH], dt_out, name="o5")

            def osb(j):
                return (p01, p23, o4, o5)[min(j // 2, 2) + (1 if j == 5 else 0)], (
                    (j % 2) * BATCH if j < 4 else 0
                )

            for j in range(NCHUNK):
                compj = cpair[j // 2][:, (j % 2) * CCOLS : (j % 2 + 1) * CCOLS]
                ps = pspool.tile([128, 1024], f32, tag="ps", name=f"ps{j}")
                for h, (lo_h, hi_h) in enumerate(SEGS):
                    pieces = []
                    for t in range(KTILES):
                        lo = max(ranges[t], lo_h)
                        hi = min(ranges[t + 1], hi_h)
                        if lo < hi:
                            pieces.append((t, lo, hi))
                    # disjoint column pieces cover the bank; first starts the
                    # accumulation group, later ones land on untouched
                    # elements (per-element has_written => plain writes)
                    for i, (t, lo, hi) in enumerate(pieces):
                        kh = _kheight(t)
                        nc.tensor.matmul(
                            ps[:, lo:hi],
                            compj[:kh, t * 128 : (t + 1) * 128],
                            epack[:kh, lo:hi],
                            start=(i == 0),
                            stop=(i == len(pieces) - 1),
                        )
                dst, off = osb(j)
                if j < NCHUNK - 1:
                    # one full-chunk drain; alternate the two PSUM-capable
                    # engines chunk by chunk (cast f32->fp16 is free here)
                    if j % 2 == 0:
                        nc.vector.tensor_copy(
                            out=dst[:, off : off + BATCH], in_=ps
                        )
                    else:
                        nc.scalar.copy(dst[:, off : off + BATCH], ps)
                else:
                    # last chunk: per-segment drains on both engines so the
                    # final stores start after each half, shortening the tail
                    nc.vector.tensor_copy(out=dst[:, 0:512], in_=ps[:, 0:512])
                    nc.scalar.copy(dst[:, 512:BATCH], ps[:, 512:BATCH])
                if j == 1:
                    # chunk{0,1} store on the ACT ring: at this point the
                    # ACT queue's next instruction is chunk3's drain, and
                    # this store's wait (chunk0+1 drained) resolves earlier
                    nc.scalar.dma_start(
                        out=out_d[:, 0 : 2 * BATCH], in_=p01
                    )
                elif j == 3:
                    nc.sync.dma_start(
                        out=out_d[:, 2 * BATCH : 4 * BATCH], in_=p23
                    )
                elif j == 4:
                    nc.sync.dma_start(
                        out=out_d[:, 4 * BATCH : 5 * BATCH], in_=o4
                    )
                elif j == 5:
                    nc.sync.dma_start(
                        out=out_d[:, 5 * BATCH : 5 * BATCH + 512],
                        in_=o5[:, 0:512],
                    )
                    nc.sync.dma_start(
                        out=out_d[:, 5 * BATCH + 512 : 6 * BATCH],
                        in_=o5[:, 512:BATCH],
                    )

    nc.compile()
    return nc


def _get_nc(ranges: tuple):
    if ranges not in _NC_CACHE:
        _NC_CACHE[ranges] = _build_nc(ranges)
    return _NC_CACHE[ranges]


def _prepare(lattent_codes, object_labels, means, components):
    x = np.ascontiguousarray(np.asarray(lattent_codes), dtype=np.float32)
    labels = np.asarray(object_labels).astype(np.int64)
    means = np.ascontiguousarray(np.asarray(means), dtype=np.float32)
    comp = np.ascontiguousarray(np.asarray(components), dtype=np.float32)
    ddt = _np_dtype()

    perm = np.argsort(labels, kind="stable")
    ls = labels[perm]
    xs = x[perm]  # [B, 32]

    counts = np.bincount(ls, minlength=NOBJ)
    cum = np.concatenate([[0], np.cumsum(counts)])
    raw = [int(cum[min(OBJ_PER_KT * t, NOBJ)]) for t in range(KTILES + 1)]
    widths = [raw[t + 1] - raw[t] for t in range(KTILES)]

    # fp32r matmuls need even range starts/widths -> insert zero pad columns
    # (dst_of_src maps sorted column -> padded column; samples pushed to
    # >= BATCH fall off the device batch and are computed on the host).
    # fp16 has no such ISA restriction: no padding at all.
    pad = (lambda w: w % 2) if DTYPE == "f32r" else (lambda w: 0)
    pstart = [0]
    for t in range(KTILES):
        pstart.append(pstart[t] + widths[t] + pad(widths[t]))
    ranges = tuple(min(p, BATCH) for p in pstart[:KTILES]) + (BATCH,)
    dst_of_src = np.concatenate(
        [np.arange(widths[t]) + pstart[t] for t in range(KTILES)]
    )
    on_dev = dst_of_src < BATCH

    # host-side fallback for overflow samples (at most a few, f32r only)
    ov = np.nonzero(~on_dev)[0]
    host_rows = None
    if len(ov):
        host_rows = (
            np.einsum("bp,bpo->bo", xs[ov], comp[ls[ov]]) + means[ls[ov]]
        ).astype(np.float32)

    # Epack[(l%3)*33 + p, dst(i)] = xs[i, p]; row (l%3)*33+32 = 1.0
    band = (ls % OBJ_PER_KT).astype(np.int64)
    epack = np.zeros((KP, BATCH), ddt)
    rows = band[None, on_dev] * ROWS + np.arange(PCA)[:, None]  # [32, n_dev]
    epack[rows, dst_of_src[None, on_dev]] = xs[on_dev].T.astype(ddt)
    epack[band[on_dev] * ROWS + PCA, dst_of_src[on_dev]] = 1.0

    # augmented component table: per object 32 component rows + 1 mean row
    m2 = np.concatenate([comp, means[:, None, :]], axis=1)  # [20, 33, OUT]
    m2 = m2.reshape(NOBJ * ROWS, OUT_DIM)

    in_maps = []
    CCOLS = KTILES * 128
    for c in range(NCORES):
        sl = slice(c * SLICE, (c + 1) * SLICE)
        arr = np.zeros((KP, NCHUNK, KTILES, 128), ddt)
        for t in range(KTILES):
            kh = _kheight(t)
            blk = m2[KP * t : KP * t + kh, sl]  # [kh, 768]
            arr[:kh, :, t, :] = blk.reshape(kh, NCHUNK, 128).astype(ddt)
        comp_host = np.ascontiguousarray(arr.reshape(KP, NCHUNK * CCOLS))
        in_maps.append({"comp": comp_host, "epack": epack})
    return in_maps, ranges, perm, dst_of_src, on_dev, host_rows


def _assemble(results, perm, dst_of_src, on_dev, host_rows):
    out_s = np.empty((BATCH, OUT_DIM), np.float32)
    for c in range(NCORES):
        # de-interleave [128, NCHUNK*1024] -> [NCHUNK*128, 1024] = out_T
        out_t = (
            results[c]["out"]
            .astype(np.float32)
            .reshape(128, NCHUNK, BATCH)
            .transpose(1, 0, 2)
            .reshape(SLICE, BATCH)
        )
        out_s[on_dev, c * SLICE : (c + 1) * SLICE] = out_t.T[dst_of_src[on_dev]]
    if host_rows is not None:
        out_s[~on_dev] = host_rows
    out = np.empty_like(out_s)
    out[perm] = out_s
    return out


def run(inputs: dict, trace: bool = False):
    """Run on hardware; returns (full output, BassKernelResults)."""
    from concourse.bass_utils import run_bass_kernel_spmd

    in_maps, ranges, perm, dst_of_src, on_dev, host_rows = _prepare(**inputs)
    nc = _get_nc(ranges)
    res = run_bass_kernel_spmd(
        nc, in_maps, core_ids=list(range(NCORES)), trace=trace
    )
    return _assemble(res.results, perm, dst_of_src, on_dev, host_rows), res


def kernel(lattent_codes, object_labels, means, components) -> np.ndarray:
    out, _ = run(
        {
            "lattent_codes": lattent_codes,
            "object_labels": object_labels,
            "means": means,
            "components": components,
        }
    )
    return out


# revision 5
# speedup vs baseline: 1.0284x; 1.0284x over previous
"""Trainium2 Bass kernel for nn_DistanceDecoder (moe_routing).

reference:
    comp_b  = components[object_labels]            # [B, 32, 6144]
    mean_b  = means[object_labels]                 # [B, 6144]
    out     = einsum('bp,bpo->bo', lattent, comp_b) + mean_b

Strategy (8 NeuronCores):
  * Shard OUT_DIM (6144) 8-ways -> each core owns a 768-wide column slice
    and the full batch.  Per-core HBM traffic is then ~2.8 MB (its own
    fp16 slice of the PCA table + 1.5 MB fp16 output) instead of the
    18 MB the batch-parallel split would need.
  * On the host, stable-sort the batch by label (MoE dispatch) and append
    the per-object mean as a 33rd row of each object's [32, 768] component
    block with a matching constant-1.0 row in the latent matrix, so gather
    + vecmat + mean-add is a single block-banded matmul
        out_T[768, 1024] = C2aug^T @ Epack
    over 7 K-tiles of 3 objects (K = 3*33 = 99 rows).  After the sort,
    each K-tile's samples form one contiguous column range, baked into the
    instruction stream as matmul free-dim offsets.
  * Matmul operands are fp16 (~10-bit-mantissa rounding, full-rate PE,
    half the DMA bytes).  The output is also stored fp16 (cast during the
    PSUM->SBUF drain, upcast on the host): adds ~2e-4 rel err and halves
    the store traffic.
  * Profiling showed HBM READS are the wall (~90-100 GB/s per HWDGE ring
    vs ~400 GB/s for writes), and strided DRAM sources make it worse, so:
    (a) the host packs comp so every per-chunk load reads one fully
    CONTIGUOUS 177 KB block ([NCHUNK*KP, CCOLS] layout), (b) loads are
    split across both HWDGE rings (sync: chunks 0,1,4; scalar: epack and
    chunks 2,3,5 - each ring's FIFO drains transfers in order, so two
    rings double read throughput), (c) all loads are enqueued before any
    store on the same ring.
  * Stores are per-chunk (contiguous 256 KB in [SLICE, BATCH] layout),
    alternating rings, each emitted right after its chunk's PSUM drain so
    its semaphore wait is already resolved and cannot head-of-line block
    a later drain on the ACT queue.  The last chunk drains and stores per
    512-segment to shorten the final drain before the exit barrier.
  * PSUM drains are one [128,1024] f32->fp16 copy per chunk (2 PSUM
    banks, pool of 4), alternating DVE / ACT.
  * DD_DTYPE=f32r swaps in fp32r (full fp32 operand bits, f32 output);
    fp32r matmuls then require even range starts/widths, fixed by zero
    pad columns, with samples pushed past column 1024 computed on the
    host.
  * Host applies the inverse permutation / column concat at the end.
"""

import os

import numpy as np

BATCH = 1024
PCA = 32
ROWS = PCA + 1             # 32 components + 1 mean row per object
OUT_DIM = 6144
NOBJ = 20
NCORES = 8
SLICE = OUT_DIM // NCORES  # 768
NCHUNK = SLICE // 128      # 6 chunks of 128 output rows (out_T partitions)
OBJ_PER_KT = 3             # objects per K-tile -> K = 3*33 = 99 <= 128
KTILES = (NOBJ + OBJ_PER_KT - 1) // OBJ_PER_KT  # 7
KP = OBJ_PER_KT * ROWS     # 99 partitions per K-tile
SEGS = [(0, 512), (512, 1024)]  # PSUM bank segments

DTYPE = os.environ.get("DD_DTYPE", "fp16")  # "fp16" | "f32r"

_NC_CACHE: dict = {}


def _kheight(t: int) -> int:
    return (min(OBJ_PER_KT * (t + 1), NOBJ) - OBJ_PER_KT * t) * ROWS


def _np_dtype():
    return np.float16 if DTYPE == "fp16" else np.float32


def _build_nc(ranges: tuple):
    """Build + compile the single-core Bass program (SPMD across 8 cores).

    ranges: KTILES+1 ints; ranges[t]..ranges[t+1] is the sorted-batch column
    range whose labels fall in objects [3t, 3t+3) — baked into the
    instruction stream as matmul free-dim offsets.
    """
    import concourse.mybir as mybir
    from concourse import bacc
    from concourse.tile import TileContext

    dt_in = mybir.dt.float16 if DTYPE == "fp16" else mybir.dt.float32r
    dt_out = mybir.dt.float16 if DTYPE == "fp16" else mybir.dt.float32
    f32 = mybir.dt.float32

    nc = bacc.Bacc("TRN2", target_bir_lowering=False, debug=False)

    # comp packed per chunk: rows [j*KP, (j+1)*KP) = chunk j, contiguous
    CCOLS = KTILES * 128
    comp_d = nc.dram_tensor(
        "comp", [NCHUNK * KP, CCOLS], dt_in, kind="ExternalInput"
    )
    epack_d = nc.dram_tensor("epack", [KP, BATCH], dt_in, kind="ExternalInput")
    out_d = nc.dram_tensor("out", [SLICE, BATCH], dt_out, kind="ExternalOutput")

    with TileContext(nc) as tc:
        with (
            tc.tile_pool(name="const", bufs=1) as cpool,
            tc.tile_pool(name="outp", bufs=1) as opool,
            tc.tile_pool(name="ps", bufs=4, space="PSUM") as pspool,
        ):
            # loads split across both HWDGE rings, all enqueued before any
            # store on the same ring (per-ring FIFO): sync c0,c1,c4;
            # scalar ep,c2,c3,c5
            comps = [
                cpool.tile([KP, CCOLS], dt_in, name=f"comp{j}")
                for j in range(NCHUNK)
            ]

            def load(j, eng):
                eng.dma_start(
                    out=comps[j], in_=comp_d[j * KP : (j + 1) * KP, :]
                )

            load(0, nc.sync)
            epack = cpool.tile([KP, BATCH], dt_in)
            nc.scalar.dma_start(out=epack, in_=epack_d[:, :])
            load(1, nc.sync)
            load(2, nc.scalar)
            load(3, nc.scalar)
            load(4, nc.sync)
            load(5, nc.scalar)

            for j in range(NCHUNK):
                compj = comps[j]
                last = j == NCHUNK - 1
                out_sb = opool.tile(
                    [128, BATCH], dt_out, name=f"osb{j}"
                )
                ps = pspool.tile([128, 1024], f32, tag="ps", name=f"ps{j}")
                for h, (lo_h, hi_h) in enumerate(SEGS):
                    pieces = []
                    for t in range(KTILES):
                        lo = max(ranges[t], lo_h)
                        hi = min(ranges[t + 1], hi_h)
                        if lo < hi:
                            pieces.append((t, lo, hi))
                    # disjoint column pieces cover the bank; first starts the
                    # accumulation group, later ones land on untouched
                    # elements (per-element has_written => plain writes)
                    for i, (t, lo, hi) in enumerate(pieces):
                        kh = _kheight(t)
                        nc.tensor.matmul(
                            ps[:, lo:hi],
                            compj[:kh, t * 128 : (t + 1) * 128],
                            epack[:kh, lo:hi],
                            start=(i == 0),
                            stop=(i == len(pieces) - 1),
                        )
                    if last:
                        # final chunk: per-segment drains on both engines +
                        # per-segment stores for the shortest tail
                        if h == 0:
                            nc.vector.tensor_copy(
                                out=out_sb[:, 0:512], in_=ps[:, 0:512]
                            )
                            nc.sync.dma_start(
                                out=out_d[j * 128 : (j + 1) * 128, 0:512],
                                in_=out_sb[:, 0:512],
                            )
                        else:
                            nc.scalar.copy(out_sb[:, 512:BATCH], ps[:, 512:BATCH])
                            nc.scalar.dma_start(
                                out=out_d[j * 128 : (j + 1) * 128, 512:BATCH],
                                in_=out_sb[:, 512:BATCH],
                            )
                if not last:
                    # one full-chunk [128,1024] drain (f32->fp16 cast here),
                    # alternating the two PSUM-capable engines; store follows
                    # on the matching ring so its wait is already resolved
                    if j % 2 == 0:
                        nc.vector.tensor_copy(out=out_sb, in_=ps)
                        nc.sync.dma_start(
                            out=out_d[j * 128 : (j + 1) * 128, :], in_=out_sb
                        )
                    else:
                        nc.scalar.copy(out_sb, ps)
                        nc.scalar.dma_start(
                            out=out_d[j * 128 : (j + 1) * 128, :], in_=out_sb
                        )

    nc.compile()
    return nc


def _get_nc(ranges: tuple):
    if ranges not in _NC_CACHE:
        _NC_CACHE[ranges] = _build_nc(ranges)
    return _NC_CACHE[ranges]


def _prepare(lattent_codes, object_labels, means, components):
    x = np.ascontiguousarray(np.asarray(lattent_codes), dtype=np.float32)
    labels = np.asarray(object_labels).astype(np.int64)
    means = np.ascontiguousarray(np.asarray(means), dtype=np.float32)
    comp = np.ascontiguousarray(np.asarray(components), dtype=np.float32)
    ddt = _np_dtype()

    perm = np.argsort(labels, kind="stable")
    ls = labels[perm]
    xs = x[perm]  # [B, 32]

    counts = np.bincount(ls, minlength=NOBJ)
    cum = np.concatenate([[0], np.cumsum(counts)])
    raw = [int(cum[min(OBJ_PER_KT * t, NOBJ)]) for t in range(KTILES + 1)]
    widths = [raw[t + 1] - raw[t] for t in range(KTILES)]

    # fp32r matmuls need even range starts/widths -> insert zero pad columns
    # (dst_of_src maps sorted column -> padded column; samples pushed to
    # >= BATCH fall off the device batch and are computed on the host).
    # fp16 has no such ISA restriction: no padding at all.
    pad = (lambda w: w % 2) if DTYPE == "f32r" else (lambda w: 0)
    pstart = [0]
    for t in range(KTILES):
        pstart.append(pstart[t] + widths[t] + pad(widths[t]))
    ranges = tuple(min(p, BATCH) for p in pstart[:KTILES]) + (BATCH,)
    dst_of_src = np.concatenate(
        [np.arange(widths[t]) + pstart[t] for t in range(KTILES)]
    )
    on_dev = dst_of_src < BATCH

    # host-side fallback for overflow samples (at most a few, f32r only)
    ov = np.nonzero(~on_dev)[0]
    host_rows = None
    if len(ov):
        host_rows = (
            np.einsum("bp,bpo->bo", xs[ov], comp[ls[ov]]) + means[ls[ov]]
        ).astype(np.float32)

    # Epack[(l%3)*33 + p, dst(i)] = xs[i, p]; row (l%3)*33+32 = 1.0
    band = (ls % OBJ_PER_KT).astype(np.int64)
    epack = np.zeros((KP, BATCH), ddt)
    rows = band[None, on_dev] * ROWS + np.arange(PCA)[:, None]  # [32, n_dev]
    epack[rows, dst_of_src[None, on_dev]] = xs[on_dev].T.astype(ddt)
    epack[band[on_dev] * ROWS + PCA, dst_of_src[on_dev]] = 1.0

    # augmented component table: per object 32 component rows + 1 mean row
    m2 = np.concatenate([comp, means[:, None, :]], axis=1)  # [20, 33, OUT]
    m2 = m2.reshape(NOBJ * ROWS, OUT_DIM)

    in_maps = []
    CCOLS = KTILES * 128
    for c in range(NCORES):
        sl = slice(c * SLICE, (c + 1) * SLICE)
        arr = np.zeros((KP, NCHUNK, KTILES, 128), ddt)
        for t in range(KTILES):
            kh = _kheight(t)
            blk = m2[KP * t : KP * t + kh, sl]  # [kh, 768]
            arr[:kh, :, t, :] = blk.reshape(kh, NCHUNK, 128).astype(ddt)
        # pack per chunk: [NCHUNK*KP, CCOLS], chunk j contiguous
        comp_host = np.ascontiguousarray(
            arr.transpose(1, 0, 2, 3).reshape(NCHUNK * KP, KTILES * 128)
        )
        in_maps.append({"comp": comp_host, "epack": epack})
    return in_maps, ranges, perm, dst_of_src, on_dev, host_rows


def _assemble(results, perm, dst_of_src, on_dev, host_rows):
    out_s = np.empty((BATCH, OUT_DIM), np.float32)
    for c in range(NCORES):
        out_s[on_dev, c * SLICE : (c + 1) * SLICE] = (
            results[c]["out"].astype(np.float32).T[dst_of_src[on_dev]]
        )
    if host_rows is not None:
        out_s[~on_dev] = host_rows
    out = np.empty_like(out_s)
    out[perm] = out_s
    return out


def run(inputs: dict, trace: bool = False):
    """Run on hardware; returns (full output, BassKernelResults)."""
    from concourse.bass_utils import run_bass_kernel_spmd

    in_maps, ranges, perm, dst_of_src, on_dev, host_rows = _prepare(**inputs)
    nc = _get_nc(ranges)
    res = run_bass_kernel_spmd(
        nc, in_maps, core_ids=list(range(NCORES)), trace=trace
    )
    return _assemble(res.results, perm, dst_of_src, on_dev, host_rows), res


def kernel(lattent_codes, object_labels, means, components) -> np.ndarray:
    out, _ = run(
        {
            "lattent_codes": lattent_codes,
            "object_labels": object_labels,
            "means": means,
            "components": components,
        }
    )
    return out
